# revision 1
# baseline (speedup 1.0000x reference)
"""Trainium2 Bass kernel for nn_AttentionBlock (GroupNorm + 8-head self-attention).

Data-parallel over batch: 8 batch elements -> 8 NeuronCores, one each.

v4:
  - x / qkv_w / proj_w host-cast to bf16; consts packed into 2 DMAs issued
    after x (x lands first)
  - single ACT table load (Exp); GroupNorm rsqrt via DVE reciprocal + Quake
    seed + Newton (no extra tables)
  - PE warm-up matmuls gated on x arrival (HAM warm before QKV)
  - rowsum reciprocal broadcast via K=1 fp32r matmuls into PSUM (no DRAM
    roundtrip); attention pairs software-pipelined across boundaries
  - proj k=0..2 + bias (rank-1 MM) + residual precomputed during pair 3 into
    bf16 partials; tail adds k=3 + identity-matmul of the partial in PSUM,
    drained on ACT+DVE in parallel
"""

import numpy as np
import ml_dtypes

NUM_GROUPS = 32
NUM_HEADS = 8
EPS = 1e-6
C = 512
N = 1024
B = 8

_cache = {}


def _build_bass():
    import concourse.bacc as bacc
    import concourse.bass as bass_mod
    import concourse.mybir as mybir
    import concourse.tile as tile

    fp32 = mybir.dt.float32
    fp32r = mybir.dt.float32r
    int32 = mybir.dt.int32
    bf16 = mybir.dt.bfloat16
    AF = mybir.ActivationFunctionType
    OP = mybir.AluOpType

    nc = bacc.Bacc("TRN2", target_bir_lowering=False, debug=False)

    x_d = nc.dram_tensor("x16", [C, N], bf16, kind="ExternalInput")
    w_d = nc.dram_tensor("w16", [C, 3 * C], bf16, kind="ExternalInput")
    p_d = nc.dram_tensor("p16", [C, C], bf16, kind="ExternalInput")
    pbT_d = nc.dram_tensor("pbT16", [4, 128], bf16, kind="ExternalInput")
    id_d = nc.dram_tensor("ident16", [128, 128], bf16, kind="ExternalInput")
    sel_d = nc.dram_tensor("selmat", [128, 2, 128], fp32, kind="ExternalInput")
    cm_d = nc.dram_tensor("cmain", [128, 144], fp32, kind="ExternalInput")
    qkvb_d = nc.dram_tensor("qkv_b", [3 * C], fp32, kind="ExternalInput")
    GT_d = nc.dram_tensor("GTmat", [32, 4, 128], fp32, kind="ExternalInput")
    y_d = nc.dram_tensor("y", [C, N], fp32, kind="ExternalOutput")

    with tile.TileContext(nc) as tc:
        with (
            tc.tile_pool(name="const", bufs=1) as const,
            tc.tile_pool(name="work", bufs=1) as work,
            tc.tile_pool(name="ppool", bufs=2) as ppool,
            tc.tile_pool(name="ypool", bufs=3) as ypool,
            tc.tile_pool(name="pss", bufs=2, space="PSUM") as pss,     # 2x[128,1024]
            tc.tile_pool(name="psav", bufs=1, space="PSUM") as psav,   # [128,1024]
            tc.tile_pool(name="psr", bufs=1, space="PSUM") as psr,     # [128,512]
            tc.tile_pool(name="psq", bufs=1, space="PSUM") as psq,     # [128,512]
        ):
            # ---------------- ACT table pre-warm (exp set) ------------------
            warm = const.tile([32, 1], fp32, tag="warm")
            nc.vector.memset(warm[:], 1.0)
            nc.scalar.activation(warm[:], warm[:], AF.Exp, scale=1.0)

            # ---------------- input DMA: x first ----------------------------
            x_bf = work.tile([128, 4, N], bf16, tag="x")
            x_engs = [nc.sync, nc.scalar, nc.gpsimd, nc.sync]
            for j in range(4):
                x_engs[j].dma_start(
                    x_bf[:, j, :], x_d.ap().rearrange("(j p) n -> j p n", p=128)[j]
                )

            # consts (packed DMAs + broadcasts), then weights
            GT_sb = const.tile([32, 4, 128], fp32, tag="GT")
            nc.sync.dma_start(GT_sb[:], GT_d.ap())
            cm_sb = const.tile([128, 144], fp32, tag="cm")
            nc.sync.dma_start(cm_sb[:], cm_d.ap())
            vb_src = qkvb_d.ap()[2 * C : 3 * C]
            vb_bcast_ap = bass_mod.AP(
                tensor=vb_src.tensor, offset=vb_src.offset, ap=[[0, 128], [1, C]]
            )
            vb_bc = const.tile([128, C], fp32, tag="vbbc")
            nc.gpsimd.dma_start(vb_bc[:], vb_bcast_ap)
            pbT_sb = const.tile([1, 4, 128], bf16, tag="pbT")
            nc.gpsimd.dma_start(pbT_sb[:], pbT_d.ap()[None, :, :])

            # weights: Q/K pieces on sync (needed first), V + proj on gpsimd
            w_bf = work.tile([128, 4, 3 * C], bf16, tag="wbf")
            for lo, hi in [(0, 512), (512, 1024)]:
                for j in range(4):
                    nc.sync.dma_start(
                        w_bf[:, j, lo:hi],
                        w_d.ap().rearrange("(j p) o -> j p o", p=128)[j, :, lo:hi],
                    )
            sel_f = const.tile([128, 2, 128], fp32, tag="self")
            nc.sync.dma_start(sel_f[:], sel_d.ap())
            for j in range(4):
                nc.gpsimd.dma_start(
                    w_bf[:, j, 1024:1536],
                    w_d.ap().rearrange("(j p) o -> j p o", p=128)[j, :, 1024:1536],
                )
            p_bf = work.tile([128, 4, C], bf16, tag="pbf")
            for j in range(4):
                nc.gpsimd.dma_start(
                    p_bf[:, j, :], p_d.ap().rearrange("(j p) o -> j p o", p=128)[j]
                )
            id_sb = const.tile([128, 128], bf16, tag="ident")
            nc.gpsimd.dma_start(id_sb[:], id_d.ap())
            sel_r = const.tile([128, 2, 128], fp32r, tag="selr")

            # const views
            G_sb = cm_sb[:, 0:128].rearrange("p (j g) -> p j g", j=4)
            nw_sb = cm_sb[:, 128:132]
            nb_sb = cm_sb[:, 132:136]
            qb_sb = cm_sb[:, 136:144]

            ones_bf = const.tile([128, 1], bf16, tag="ones")
            nc.vector.memset(ones_bf[:], 1.0)
            ones_row = const.tile([1, 512], bf16, tag="onesrow")
            nc.vector.memset(ones_row[:], 1.0)

            # ---------------- PE warm-up (HAM) on x chunk 0 -----------------
            junk = psq.tile([128, 512], fp32, tag="q", name="warmup")
            for _ in range(11):
                nc.tensor.matmul(
                    junk[:], x_bf[:, 0, 0:128], x_bf[:, 0, 0:512],
                    start=True, stop=True,
                )

            # ---------------- groupnorm ----------------
            stats = work.tile([128, 4, 2, 6], fp32, tag="stats")
            mv = work.tile([128, 4, 2], fp32, tag="mv")
            for j in range(4):
                for u in range(2):
                    nc.vector.bn_stats(
                        stats[:, j, u, :], x_bf[:, j, u * 512 : u * 512 + 512]
                    )
                nc.vector.bn_aggr(mv[:, j, :], stats[:, j, :, :])
            ssq = work.tile([128, 4, 2], fp32, tag="ssq")
            nc.vector.tensor_copy(ssq[:, :, 0], mv[:, :, 0])
            nc.vector.tensor_tensor(ssq[:, :, 1], mv[:, :, 0], mv[:, :, 0], op=OP.mult)
            nc.vector.tensor_tensor(ssq[:, :, 1], ssq[:, :, 1], mv[:, :, 1], op=OP.add)
            ps_g = psr.tile([32, 2], fp32, tag="r")
            for j in range(4):
                nc.tensor.matmul(
                    ps_g[:], G_sb[:, j, :], ssq[:, j, :], start=(j == 0), stop=(j == 3)
                )
            st2 = work.tile([32, 2], fp32, tag="st2")
            nc.vector.tensor_copy(st2[:, 0:1], ps_g[:, 0:1])
            var = work.tile([32, 1], fp32, tag="var")
            nc.vector.tensor_tensor(var[:], st2[:, 0:1], st2[:, 0:1], op=OP.mult)
            nc.vector.tensor_tensor(var[:], ps_g[:, 1:2], var[:], op=OP.subtract)
            nc.vector.tensor_scalar(var[:], var[:], float(EPS), None, op0=OP.add)

            # PE keep-warm batch A: bf16, gated on ssq via a tiny cast
            gate_a = work.tile([128, 2], bf16, tag="gatea")
            nc.vector.tensor_copy(gate_a[:], ssq[:, 0, 0:2])
            junkA = psq.tile([128, 512], fp32, tag="q", name="warmA")
            for _ in range(5):
                nc.tensor.matmul(
                    junkA[0:2, :], gate_a[:], x_bf[:, 0, 0:512],
                    start=True, stop=True,
                )
            # broadcast mean while rstd is still being computed
            ps_bc = psr.tile([128, 4, 2], fp32, tag="r")
            for j in range(4):
                nc.tensor.matmul(
                    ps_bc[:, j, 0:1], GT_sb[:, j, :], st2[:, 0:1],
                    start=True, stop=True,
                )

            # rstd = 1/sqrt(var): reciprocal, Quake sqrt seed, 1 Heron iter
            rcp = work.tile([32, 1], fp32, tag="rcp")
            nc.vector.reciprocal(rcp[:], var[:])
            y_rs = work.tile([32, 1], fp32, tag="yrs")
            nc.vector.tensor_scalar(
                y_rs.bitcast(int32)[:], rcp.bitcast(int32)[:],
                1, None, op0=OP.logical_shift_right,
            )
            nc.vector.tensor_scalar(
                y_rs.bitcast(int32)[:], y_rs.bitcast(int32)[:],
                0x1FBD1DF6, None, op0=OP.add,
            )
            t_rs = work.tile([32, 1], fp32, tag="trs")
            nc.vector.tensor_tensor(t_rs[:], var[:], y_rs[:], op=OP.mult)
            nc.vector.tensor_tensor(t_rs[:], t_rs[:], y_rs[:], op=OP.mult)
            nc.vector.tensor_scalar(t_rs[:], t_rs[:], -0.5, 1.5,
                                    op0=OP.mult, op1=OP.add)
            nc.vector.tensor_tensor(st2[:, 1:2], y_rs[:], t_rs[:], op=OP.mult)

            for j in range(4):
                nc.tensor.matmul(
                    ps_bc[:, j, 1:2], GT_sb[:, j, :], st2[:, 1:2],
                    start=True, stop=True,
                )
            ab = work.tile([128, 4, 2], fp32, tag="ab")
            nc.vector.tensor_tensor(ab[:, :, 0], ps_bc[:, :, 1], nw_sb[:], op=OP.mult)
            nc.vector.tensor_tensor(ab[:, :, 1], ps_bc[:, :, 0], ab[:, :, 0], op=OP.mult)
            nc.vector.tensor_tensor(ab[:, :, 1], nb_sb[:], ab[:, :, 1], op=OP.subtract)

            # PE keep-warm batch C: bf16, gated on ab via a tiny cast
            gate_c = work.tile([128, 2], bf16, tag="gatec")
            nc.vector.tensor_copy(gate_c[:], ab[:, 0, :])
            junkC = psq.tile([128, 512], fp32, tag="q", name="warmC")
            for _ in range(4):
                nc.tensor.matmul(
                    junkC[0:2, :], gate_c[:], x_bf[:, 0, 0:512],
                    start=True, stop=True,
                )

            xn_bf = work.tile([128, 4, N], bf16, tag="xn")
            for j in range(4):
                nc.vector.tensor_scalar(
                    xn_bf[:, j, :], x_bf[:, j, :],
                    ab[:, j, 0:1], ab[:, j, 1:2],
                    op0=OP.mult, op1=OP.add,
                )

            # rowsum PSUM bank: memset once (rows outside {0,32,64,96} stay 1.0)
            ps_r = psr.tile([128, 512], fp32, tag="r")
            nc.vector.memset(ps_r[:], 1.0)

            # ---------------- QKV emitters ----------------
            QK_bf = work.tile([128, 8, N], bf16, tag="QK")
            VT_bf = work.tile([128, 8, C], bf16, tag="VT")

            def emit_qk_full(oc, drain_on_act=False):
                ps_qk = pss.tile([128, 1024], fp32, tag="s", name=f"qk{oc}")
                for nu in range(2):
                    for k in range(4):
                        nc.tensor.matmul(
                            ps_qk[:, nu * 512 : nu * 512 + 512],
                            w_bf[:, k, oc * 128 : oc * 128 + 128],
                            xn_bf[:, k, nu * 512 : nu * 512 + 512],
                            start=(k == 0),
                            stop=(k == 3),
                        )
                if drain_on_act:
                    # half drains: lets s_unit(0,0,0) start off the first half
                    for nu in range(2):
                        nc.scalar.activation(
                            QK_bf[:, oc, nu * 512 : nu * 512 + 512],
                            ps_qk[:, nu * 512 : nu * 512 + 512], AF.Identity,
                            bias=qb_sb[:, oc : oc + 1], scale=1.0,
                        )
                else:
                    nc.vector.tensor_scalar(
                        QK_bf[:, oc, :], ps_qk[:], qb_sb[:, oc : oc + 1], None,
                        op0=OP.add,
                    )

            def emit_qk_half(oc, nu):
                ps_qk = psq.tile([128, 512], fp32, tag="q", name=f"qk{oc}h{nu}")
                for k in range(4):
                    nc.tensor.matmul(
                        ps_qk[:],
                        w_bf[:, k, oc * 128 : oc * 128 + 128],
                        xn_bf[:, k, nu * 512 : nu * 512 + 512],
                        start=(k == 0),
                        stop=(k == 3),
                    )
                nc.vector.tensor_scalar(
                    QK_bf[:, oc, nu * 512 : nu * 512 + 512],
                    ps_qk[:],
                    qb_sb[:, oc : oc + 1],
                    None,
                    op0=OP.add,
                )

            def emit_v(mc):
                ps_v = psq.tile([128, 512], fp32, tag="q", name=f"v{mc}")
                for k in range(4):
                    nc.tensor.matmul(
                        ps_v[:],
                        xn_bf[:, k, mc * 128 : mc * 128 + 128],
                        w_bf[:, k, 2 * C : 3 * C],
                        start=(k == 0),
                        stop=(k == 3),
                    )
                nc.vector.tensor_tensor(VT_bf[:, mc, :], ps_v[:], vb_bc[:], op=OP.add)

            # ---------------- attention state ----------------
            rr = work.tile([128, 4, 512], fp32, tag="rr")
            rr_r = work.tile([128, 4, 512], fp32r, tag="rrr")
            R_sb = work.tile([128, 4, N], fp32, tag="R")
            att_un = work.tile([128, 4, N], fp32, tag="attun")
            att_bf = work.tile([128, 4, N], bf16, tag="attbf")
            part_bf = work.tile([128, 4, N], bf16, tag="part")

            def s_unit(p, s, nu):
                u = pss.tile([128, 1024], fp32, tag="s", name=f"s_p{p}s{s}n{nu}")
                for e in range(2):
                    nc.tensor.matmul(
                        u[:, e * 512 : e * 512 + 512],
                        QK_bf[e * 64 : e * 64 + 64, 4 + p, s * 128 : s * 128 + 128],
                        QK_bf[e * 64 : e * 64 + 64, p, nu * 512 : nu * 512 + 512],
                        start=True,
                        stop=True,
                        skip_group_check=True,
                    )
                return u

            def emit_proj_part(oc, nu):
                # proj partial k=0..2 + bias (rank-1); drain adds residual
                ps_o = psq.tile([128, 512], fp32, tag="q", name=f"pp{oc}n{nu}")
                for k in range(3):
                    nc.tensor.matmul(
                        ps_o[:],
                        p_bf[:, k, oc * 128 : oc * 128 + 128],
                        att_bf[:, k, nu * 512 : nu * 512 + 512],
                        start=(k == 0),
                        stop=False,
                    )
                nc.tensor.matmul(
                    ps_o[:],
                    pbT_sb[0:1, oc, :],
                    ones_row[0:1, :],
                    start=False,
                    stop=True,
                )
                nc.vector.tensor_tensor(
                    part_bf[:, oc, nu * 512 : nu * 512 + 512],
                    ps_o[:],
                    x_bf[:, oc, nu * 512 : nu * 512 + 512],
                    op=OP.add,
                )

            fillers = {
                (0, 0): [lambda: emit_v(3)],
                (0, 1): [lambda: emit_v(4)],
                (0, 2): [lambda: emit_v(5)],
                (0, 3): [lambda: emit_v(6)],
                (0, 4): [lambda: emit_v(7)],
                (0, 5): [lambda: emit_qk_half(1, 0)],
                (0, 6): [lambda: emit_qk_half(1, 1)],
                (1, 1): [lambda: emit_qk_half(5, 1)],
                (1, 2): [lambda: emit_qk_half(2, 0)],
                (1, 3): [lambda: emit_qk_half(2, 1)],
                (1, 4): [lambda: emit_qk_half(6, 0)],
                (1, 5): [lambda: emit_qk_half(6, 1)],
                (2, 1): [lambda: emit_qk_half(3, 0)],
                (2, 2): [lambda: emit_qk_half(3, 1)],
                (2, 3): [lambda: emit_qk_half(7, 0)],
                (2, 4): [lambda: emit_qk_half(7, 1)],
                (3, 2): [lambda: emit_proj_part(0, 0)],
                (3, 3): [lambda: emit_proj_part(0, 1)],
                (3, 4): [lambda: emit_proj_part(1, 0)],
                (3, 5): [lambda: emit_proj_part(1, 1)],
                (3, 6): [lambda: emit_proj_part(2, 0), lambda: emit_proj_part(2, 1)],
                (3, 7): [lambda: emit_proj_part(3, 0), lambda: emit_proj_part(3, 1)],
            }

            # ---------------- QKV head + first S units ----------------
            emit_qk_full(4, drain_on_act=True)
            emit_qk_full(0, drain_on_act=True)
            units = [s_unit(0, 0, 0), s_unit(0, 0, 1)]
            emit_v(0)
            emit_v(1)
            emit_v(2)
            emit_qk_half(5, 0)
            nc.vector.tensor_copy(sel_r[:], sel_f[:])

            # ---------------- pipelined pair loop ----------------
            for p in range(4):
                P_bf = ppool.tile([128, 8, 2, 2, 512], bf16, tag="P")
                ps_av = psav.tile([128, 1024], fp32, tag="av")
                for s in range(8):
                    for nu in range(2):
                        nc.scalar.activation(
                            P_bf[:, s, nu, :, :],
                            units[nu][:].rearrange("q (h n) -> q h n", h=2),
                            AF.Exp,
                            scale=0.125,
                        )
                    if s < 7:
                        units = [s_unit(p, s + 1, 0), s_unit(p, s + 1, 1)]
                    elif p < 3:
                        units = [s_unit(p + 1, 0, 0), s_unit(p + 1, 0, 1)]
                    for nu in range(2):
                        for e in range(2):
                            row = e * 64 + nu * 32
                            nc.tensor.matmul(
                                ps_r[row : row + 1, :],
                                ones_bf[:],
                                P_bf[:, s, nu, e, :],
                                start=(s == 0),
                                stop=(s == 7),
                                tile_position=(0, row),
                                skip_group_check=True,
                            )
                    for nu in range(2):
                        for e in range(2):
                            nc.tensor.matmul(
                                ps_av[e * 64 : e * 64 + 64, nu * 512 : nu * 512 + 512],
                                VT_bf[:, s, p * 128 + e * 64 : p * 128 + e * 64 + 64],
                                P_bf[:, s, nu, e, :],
                                start=(s == 0),
                                stop=(s == 7),
                                skip_group_check=True,
                            )
                    for th in fillers.get((p, s), ()):
                        th()

                # ---- pair tail ----
                nc.vector.reciprocal_approx_fast(rr[:, p, :], ps_r[:])
                nc.vector.tensor_copy(rr_r[:, p, :], rr[:, p, :])
                # drain AV accumulator to SBUF (frees psav for R broadcast);
                # pair 3 splits drains across ACT+DVE (latency-critical)
                if p < 3:
                    nc.vector.tensor_copy(att_un[:, p, :], ps_av[:])
                else:
                    nc.scalar.activation(att_un[:, p, 0:512], ps_av[:, 0:512],
                                         AF.Identity, bias=0.0, scale=1.0)
                    nc.vector.tensor_copy(att_un[:, p, 512:1024],
                                          ps_av[:, 512:1024])
                # broadcast rr rows across partitions: R = SEL^T @ rr (fp32r)
                ps_R = psav.tile([128, 1024], fp32, tag="av", name=f"R{p}")
                for nu in range(2):
                    nc.tensor.matmul(
                        ps_R[:, nu * 512 : nu * 512 + 512],
                        sel_r[:, nu, :],
                        rr_r[:, p, :],
                        start=True,
                        stop=True,
                    )
                if p == 3:
                    # keep PE busy through the tail chain so HAM stays warm
                    junk3 = psq.tile([128, 512], fp32, tag="q", name="warmup3")
                    for _ in range(6):
                        nc.tensor.matmul(
                            junk3[:], x_bf[:, 0, 0:128], x_bf[:, 0, 0:512],
                            start=True, stop=True,
                        )
                    nc.scalar.activation(R_sb[:, p, 0:512], ps_R[:, 0:512],
                                         AF.Identity, bias=0.0, scale=1.0)
                    nc.vector.tensor_copy(R_sb[:, p, 512:1024],
                                          ps_R[:, 512:1024])
                else:
                    nc.vector.tensor_copy(R_sb[:, p, :], ps_R[:])
                # normalize off-path: pairs 0-1 on Pool; pairs 2-3 on DVE
                # (pair 2's result gates the pair-3 proj-partial fillers)
                if p < 2:
                    nc.gpsimd.tensor_tensor(
                        att_bf[:, p, :], att_un[:, p, :], R_sb[:, p, :], op=OP.mult
                    )
                else:
                    nc.vector.tensor_tensor(
                        att_bf[:, p, :], att_un[:, p, :], R_sb[:, p, :], op=OP.mult
                    )

            # ---------------- proj k=3 + identity(part) + output ------------
            y_engs = [nc.sync, nc.scalar, nc.gpsimd, nc.sync]

            def k3_mms(ps, oc, nu):
                nc.tensor.matmul(
                    ps,
                    p_bf[:, 3, oc * 128 : oc * 128 + 128],
                    att_bf[:, 3, nu * 512 : nu * 512 + 512],
                    start=True,
                    stop=False,
                )
                nc.tensor.matmul(
                    ps,
                    id_sb[:],
                    part_bf[:, oc, nu * 512 : nu * 512 + 512],
                    start=False,
                    stop=True,
                )

            def emit_y(oc, y_sb):
                y_engs[oc].dma_start(
                    y_d.ap().rearrange("(j p) n -> j p n", p=128)[oc], y_sb[:]
                )

            # oc0/oc1 via pss (ACT / DVE drains), oc3 via psav (DVE),
            # oc2 via psq halves (ACT) — maximizes drain parallelism
            for oc in (0, 1):
                ps_o = pss.tile([128, 1024], fp32, tag="s", name=f"k3_{oc}")
                for nu in range(2):
                    k3_mms(ps_o[:, nu * 512 : nu * 512 + 512], oc, nu)
                y_sb = ypool.tile([128, 1024], fp32, tag="y")
                if oc == 0:
                    nc.scalar.activation(y_sb[:], ps_o[:], AF.Identity,
                                         bias=0.0, scale=1.0)
                else:
                    nc.vector.tensor_copy(y_sb[:], ps_o[:])
                emit_y(oc, y_sb)
            ps_o3 = psav.tile([128, 1024], fp32, tag="av", name="k3_3")
            for nu in range(2):
                k3_mms(ps_o3[:, nu * 512 : nu * 512 + 512], 3, nu)
            y3 = ypool.tile([128, 1024], fp32, tag="y")
            nc.vector.tensor_copy(y3[:], ps_o3[:])
            emit_y(3, y3)
            y2 = ypool.tile([128, 1024], fp32, tag="y")
            for nu in range(2):
                ps_o2 = psq.tile([128, 512], fp32, tag="q", name=f"k3_2n{nu}")
                k3_mms(ps_o2[:], 2, nu)
                nc.scalar.activation(y2[:, nu * 512 : nu * 512 + 512], ps_o2[:],
                                     AF.Identity, bias=0.0, scale=1.0)
            emit_y(2, y2)

    nc.compile()
    return nc


def _get_nc(debug=False):
    if "nc" not in _cache:
        _cache["nc"] = _build_bass()
    return _cache["nc"]


def _host_inputs(x, norm_w, norm_b, qkv_w, qkv_b, proj_w, proj_b):
    bf = ml_dtypes.bfloat16
    x = np.asarray(x, dtype=np.float32).reshape(B, C, N).astype(bf)
    w16 = np.ascontiguousarray(np.asarray(qkv_w, dtype=np.float32).T).astype(bf)
    p16 = np.ascontiguousarray(np.asarray(proj_w, dtype=np.float32).T).astype(bf)
    pbT16 = np.asarray(proj_b, dtype=np.float32).reshape(4, 128).astype(bf)
    ident16 = np.eye(128, dtype=np.float32).astype(bf)
    # selmat[row, nu, ch] = 1 iff row == (ch//64)*64 + nu*32  (R broadcast)
    selmat = np.zeros((128, 2, 128), dtype=np.float32)
    for nu in range(2):
        for ch in range(128):
            selmat[(ch // 64) * 64 + nu * 32, nu, ch] = 1.0
    G = np.zeros((128, 128), dtype=np.float32)
    GT = np.zeros((32, 4, 128), dtype=np.float32)
    for j in range(4):
        for p in range(128):
            g = 8 * j + p // 16
            G[p, j * 32 + g] = 1.0 / 16.0
            GT[g, j, p] = 1.0
    cmain = np.zeros((128, 144), dtype=np.float32)
    cmain[:, 0:128] = G
    cmain[:, 128:132] = np.asarray(norm_w, dtype=np.float32).reshape(4, 128).T
    cmain[:, 132:136] = np.asarray(norm_b, dtype=np.float32).reshape(4, 128).T
    cmain[:, 136:144] = (
        np.asarray(qkv_b, dtype=np.float32)[0 : 2 * C].reshape(8, 128).T
    )
    shared = {
        "w16": w16,
        "p16": p16,
        "pbT16": pbT16,
        "ident16": ident16,
        "selmat": selmat,
        "cmain": cmain,
        "qkv_b": np.asarray(qkv_b, dtype=np.float32),
        "GTmat": GT,
    }
    in_maps = [dict(shared, x16=np.ascontiguousarray(x[i])) for i in range(B)]
    return in_maps


def kernel(x, norm_w, norm_b, qkv_w, qkv_b, proj_w, proj_b, _trace=False):
    from concourse import bass_utils

    nc = _get_nc()
    in_maps = _host_inputs(x, norm_w, norm_b, qkv_w, qkv_b, proj_w, proj_b)
    res = bass_utils.run_bass_kernel_spmd(
        nc, in_maps, core_ids=list(range(B)), trace=_trace
    )
    out = np.stack([res.results[i]["y"] for i in range(B)])
    _cache["last_result"] = res
    return out.reshape(B, C, 32, 32)



# revision 10
# speedup vs baseline: 1.0368x; 1.0368x over previous
"""Trainium2 Bass kernel for nn_AttentionBlock (GroupNorm + 8-head self-attention).

Data-parallel over batch: 8 batch elements -> 8 NeuronCores, one each.

v5 (from v4):
  - softmax exp split across engines: nu=0 tile on ACT (table exp), nu=1 tile
    on DVE via a one-instruction Schraudolph bit-hack writing bf16 directly
    (out_i16 = convert(S*A + B), bitcast to bf16) -- halves the ACT exp load
  - middle-loop vector work rebalanced: AV drains / R copies / normalize for
    pairs 0-2 moved to Pool (gpsimd); QK-half drains moved to Pool
  - proj bias rank-1 matmuls removed; bias fused into the proj-partial drain
    via scalar_tensor_tensor (psum + pb) + residual
  - x DMA split into 8 half-chunks across 4 queues; bn_stats per half; xn
    scale-add distributed across DVE/Pool/ACT
  - y output drained + DMA'd in 512-col halves across engines/queues
"""

import numpy as np
import ml_dtypes

NUM_GROUPS = 32
NUM_HEADS = 8
EPS = 1e-6
C = 512
N = 1024
B = 8

# Schraudolph bf16 exp: bf16_bits(exp(0.125*s)) ~= int16(round(s*A16 + B16))
A16 = float(2**7 / np.log(2.0) * 0.125)
B16 = float(127 * 128 - 5.7)

_cache = {}


def _build_bass():
    import concourse.bacc as bacc
    import concourse.bass as bass_mod
    import concourse.mybir as mybir
    import concourse.tile as tile

    fp32 = mybir.dt.float32
    fp32r = mybir.dt.float32r
    int32 = mybir.dt.int32
    int16 = mybir.dt.int16
    bf16 = mybir.dt.bfloat16
    AF = mybir.ActivationFunctionType
    OP = mybir.AluOpType

    nc = bacc.Bacc("TRN2", target_bir_lowering=False, debug=False)

    x_d = nc.dram_tensor("x16", [C, N], bf16, kind="ExternalInput")
    w_d = nc.dram_tensor("w16", [C, 3 * C], bf16, kind="ExternalInput")
    p_d = nc.dram_tensor("p16", [C, C], bf16, kind="ExternalInput")
    id_d = nc.dram_tensor("ident16", [128, 128], bf16, kind="ExternalInput")
    sel_d = nc.dram_tensor("selmat", [128, 2, 128], fp32, kind="ExternalInput")
    cm_d = nc.dram_tensor("cmain", [128, 148], fp32, kind="ExternalInput")
    qkvb_d = nc.dram_tensor("qkv_b", [3 * C], fp32, kind="ExternalInput")
    GT_d = nc.dram_tensor("GTmat", [32, 4, 128], fp32, kind="ExternalInput")
    y_d = nc.dram_tensor("y", [C, N], fp32, kind="ExternalOutput")

    with tile.TileContext(nc) as tc:
        with (
            tc.tile_pool(name="const", bufs=1) as const,
            tc.tile_pool(name="work", bufs=1) as work,
            tc.tile_pool(name="ppool", bufs=2) as ppool,
            tc.tile_pool(name="ypool", bufs=3) as ypool,
            tc.tile_pool(name="pss", bufs=2, space="PSUM") as pss,     # 2x[128,1024]
            tc.tile_pool(name="psav", bufs=1, space="PSUM") as psav,   # [128,1024]
            tc.tile_pool(name="psr", bufs=1, space="PSUM") as psr,     # [128,512]
            tc.tile_pool(name="psq", bufs=1, space="PSUM") as psq,     # [128,512]
        ):
            # ---------------- ACT table pre-warm (exp set) ------------------
            warm = const.tile([32, 1], fp32, tag="warm")
            nc.vector.memset(warm[:], 1.0)
            nc.scalar.activation(warm[:], warm[:], AF.Exp, scale=1.0)

            # ---------------- input DMA: x first, 8 half-chunks -------------
            x_bf = work.tile([128, 4, N], bf16, tag="x")
            x_engs = [nc.sync, nc.scalar, nc.gpsimd]
            for j in range(4):
                for u in range(2):
                    x_engs[(2 * j + u) % 3].dma_start(
                        x_bf[:, j, u * 512 : u * 512 + 512],
                        x_d.ap().rearrange("(j p) n -> j p n", p=128)[
                            j, :, u * 512 : u * 512 + 512
                        ],
                    )

            # consts (packed DMAs + broadcasts), then weights
            GT_sb = const.tile([32, 4, 128], fp32, tag="GT")
            nc.sync.dma_start(GT_sb[:], GT_d.ap())
            cm_sb = const.tile([128, 148], fp32, tag="cm")
            nc.sync.dma_start(cm_sb[:], cm_d.ap())
            vb_src = qkvb_d.ap()[2 * C : 3 * C]
            vb_bcast_ap = bass_mod.AP(
                tensor=vb_src.tensor, offset=vb_src.offset, ap=[[0, 128], [1, C]]
            )
            vb_bc = const.tile([128, C], fp32, tag="vbbc")
            nc.gpsimd.dma_start(vb_bc[:], vb_bcast_ap)

            # weights: Q/K pieces on sync (needed first), V + proj on gpsimd
            w_bf = work.tile([128, 4, 3 * C], bf16, tag="wbf")
            for lo, hi in [(0, 512), (512, 1024)]:
                for j in range(4):
                    nc.sync.dma_start(
                        w_bf[:, j, lo:hi],
                        w_d.ap().rearrange("(j p) o -> j p o", p=128)[j, :, lo:hi],
                    )
            sel_f = const.tile([128, 2, 128], fp32, tag="self")
            nc.sync.dma_start(sel_f[:], sel_d.ap())
            for j in range(4):
                nc.gpsimd.dma_start(
                    w_bf[:, j, 1024:1536],
                    w_d.ap().rearrange("(j p) o -> j p o", p=128)[j, :, 1024:1536],
                )
            p_bf = work.tile([128, 4, C], bf16, tag="pbf")
            for j in range(4):
                nc.gpsimd.dma_start(
                    p_bf[:, j, :], p_d.ap().rearrange("(j p) o -> j p o", p=128)[j]
                )
            id_sb = const.tile([128, 128], bf16, tag="ident")
            nc.gpsimd.dma_start(id_sb[:], id_d.ap())
            sel_r = const.tile([128, 2, 128], fp32r, tag="selr")

            # const views
            G_sb = cm_sb[:, 0:128].rearrange("p (j g) -> p j g", j=4)
            nw_sb = cm_sb[:, 128:132]
            nb_sb = cm_sb[:, 132:136]
            qb_sb = cm_sb[:, 136:144]
            pb_sb = cm_sb[:, 144:148]  # proj bias, [128, oc]

            ones_bf = const.tile([128, 1], bf16, tag="ones")
            nc.vector.memset(ones_bf[:], 1.0)

            # ---------------- PE warm-up (HAM) on x chunk 0 -----------------
            junk = psq.tile([128, 512], fp32, tag="q", name="warmup")
            for _ in range(11):
                nc.tensor.matmul(
                    junk[:], x_bf[:, 0, 0:128], x_bf[:, 0, 0:512],
                    start=True, stop=True,
                )

            # ---------------- groupnorm ----------------
            stats = work.tile([128, 4, 2, 6], fp32, tag="stats")
            mv = work.tile([128, 4, 2], fp32, tag="mv")
            for j in range(4):
                for u in range(2):
                    nc.vector.bn_stats(
                        stats[:, j, u, :], x_bf[:, j, u * 512 : u * 512 + 512]
                    )
                nc.vector.bn_aggr(mv[:, j, :], stats[:, j, :, :])
            ssq = work.tile([128, 4, 2], fp32, tag="ssq")
            nc.vector.tensor_copy(ssq[:, :, 0], mv[:, :, 0])
            nc.vector.tensor_tensor(ssq[:, :, 1], mv[:, :, 0], mv[:, :, 0], op=OP.mult)
            nc.vector.tensor_tensor(ssq[:, :, 1], ssq[:, :, 1], mv[:, :, 1], op=OP.add)
            ps_g = psr.tile([32, 2], fp32, tag="r")
            for j in range(4):
                nc.tensor.matmul(
                    ps_g[:], G_sb[:, j, :], ssq[:, j, :], start=(j == 0), stop=(j == 3)
                )
            st2 = work.tile([32, 2], fp32, tag="st2")
            nc.vector.tensor_copy(st2[:, 0:1], ps_g[:, 0:1])
            var = work.tile([32, 1], fp32, tag="var")
            nc.vector.tensor_tensor(var[:], st2[:, 0:1], st2[:, 0:1], op=OP.mult)
            nc.vector.tensor_tensor(var[:], ps_g[:, 1:2], var[:], op=OP.subtract)
            nc.vector.tensor_scalar(var[:], var[:], float(EPS), None, op0=OP.add)

            # PE keep-warm batch A: bf16, gated on ssq via a tiny cast
            gate_a = work.tile([128, 2], bf16, tag="gatea")
            nc.vector.tensor_copy(gate_a[:], ssq[:, 0, 0:2])
            junkA = psq.tile([128, 512], fp32, tag="q", name="warmA")
            for _ in range(5):
                nc.tensor.matmul(
                    junkA[0:2, :], gate_a[:], x_bf[:, 0, 0:512],
                    start=True, stop=True,
                )
            # broadcast mean while rstd is still being computed
            ps_bc = psr.tile([128, 4, 2], fp32, tag="r")
            for j in range(4):
                nc.tensor.matmul(
                    ps_bc[:, j, 0:1], GT_sb[:, j, :], st2[:, 0:1],
                    start=True, stop=True,
                )

            # rstd = 1/sqrt(var): reciprocal, Quake sqrt seed, 1 Heron iter
            rcp = work.tile([32, 1], fp32, tag="rcp")
            nc.vector.reciprocal(rcp[:], var[:])
            y_rs = work.tile([32, 1], fp32, tag="yrs")
            nc.vector.tensor_scalar(
                y_rs.bitcast(int32)[:], rcp.bitcast(int32)[:],
                1, None, op0=OP.logical_shift_right,
            )
            nc.vector.tensor_scalar(
                y_rs.bitcast(int32)[:], y_rs.bitcast(int32)[:],
                0x1FBD1DF6, None, op0=OP.add,
            )
            t_rs = work.tile([32, 1], fp32, tag="trs")
            nc.vector.tensor_tensor(t_rs[:], var[:], y_rs[:], op=OP.mult)
            nc.vector.tensor_tensor(t_rs[:], t_rs[:], y_rs[:], op=OP.mult)
            nc.vector.tensor_scalar(t_rs[:], t_rs[:], -0.5, 1.5,
                                    op0=OP.mult, op1=OP.add)
            nc.vector.tensor_tensor(st2[:, 1:2], y_rs[:], t_rs[:], op=OP.mult)

            for j in range(4):
                nc.tensor.matmul(
                    ps_bc[:, j, 1:2], GT_sb[:, j, :], st2[:, 1:2],
                    start=True, stop=True,
                )
            ab = work.tile([128, 4, 2], fp32, tag="ab")
            nc.vector.tensor_tensor(ab[:, :, 0], ps_bc[:, :, 1], nw_sb[:], op=OP.mult)
            nc.vector.tensor_tensor(ab[:, :, 1], ps_bc[:, :, 0], ab[:, :, 0], op=OP.mult)
            nc.vector.tensor_tensor(ab[:, :, 1], nb_sb[:], ab[:, :, 1], op=OP.subtract)

            # PE keep-warm batch C: bf16, gated on ab via a tiny cast
            gate_c = work.tile([128, 2], bf16, tag="gatec")
            nc.vector.tensor_copy(gate_c[:], ab[:, 0, :])
            junkC = psq.tile([128, 512], fp32, tag="q", name="warmC")
            for _ in range(4):
                nc.tensor.matmul(
                    junkC[0:2, :], gate_c[:], x_bf[:, 0, 0:512],
                    start=True, stop=True,
                )

            # xn = a*x + b, distributed across DVE / Pool / ACT
            xn_bf = work.tile([128, 4, N], bf16, tag="xn")
            nc.vector.tensor_scalar(
                xn_bf[:, 0, :], x_bf[:, 0, :],
                ab[:, 0, 0:1], ab[:, 0, 1:2], op0=OP.mult, op1=OP.add,
            )
            nc.gpsimd.tensor_scalar(
                xn_bf[:, 1, :], x_bf[:, 1, :],
                ab[:, 1, 0:1], ab[:, 1, 1:2], op0=OP.mult, op1=OP.add,
            )
            nc.scalar.activation(
                xn_bf[:, 2, :], x_bf[:, 2, :], AF.Identity,
                bias=ab[:, 2, 1:2], scale=ab[:, 2, 0:1],
            )
            nc.vector.tensor_scalar(
                xn_bf[:, 3, :], x_bf[:, 3, :],
                ab[:, 3, 0:1], ab[:, 3, 1:2], op0=OP.mult, op1=OP.add,
            )

            # rowsum PSUM bank: memset once (rows outside {0,32,64,96} stay 1.0)
            ps_r = psr.tile([128, 512], fp32, tag="r")
            nc.vector.memset(ps_r[:], 1.0)

            # ---------------- QKV emitters ----------------
            QK_bf = work.tile([128, 8, N], bf16, tag="QK")
            VT_bf = work.tile([128, 8, C], bf16, tag="VT")

            def emit_qk_full(oc, drain_on_act=False):
                ps_qk = pss.tile([128, 1024], fp32, tag="s", name=f"qk{oc}")
                for nu in range(2):
                    for k in range(4):
                        nc.tensor.matmul(
                            ps_qk[:, nu * 512 : nu * 512 + 512],
                            w_bf[:, k, oc * 128 : oc * 128 + 128],
                            xn_bf[:, k, nu * 512 : nu * 512 + 512],
                            start=(k == 0),
                            stop=(k == 3),
                        )
                if drain_on_act:
                    # half drains: lets s_unit(0,0,0) start off the first half
                    for nu in range(2):
                        nc.scalar.activation(
                            QK_bf[:, oc, nu * 512 : nu * 512 + 512],
                            ps_qk[:, nu * 512 : nu * 512 + 512], AF.Identity,
                            bias=qb_sb[:, oc : oc + 1], scale=1.0,
                        )
                else:
                    nc.vector.tensor_scalar(
                        QK_bf[:, oc, :], ps_qk[:], qb_sb[:, oc : oc + 1], None,
                        op0=OP.add,
                    )

            def emit_qk_half(oc, nu):
                ps_qk = psq.tile([128, 512], fp32, tag="q", name=f"qk{oc}h{nu}")
                for k in range(4):
                    nc.tensor.matmul(
                        ps_qk[:],
                        w_bf[:, k, oc * 128 : oc * 128 + 128],
                        xn_bf[:, k, nu * 512 : nu * 512 + 512],
                        start=(k == 0),
                        stop=(k == 3),
                    )
                # drains alternate ACT / DVE (Pool cannot read PSUM)
                if (oc + nu) % 2 == 0:
                    nc.scalar.activation(
                        QK_bf[:, oc, nu * 512 : nu * 512 + 512], ps_qk[:],
                        AF.Identity, bias=qb_sb[:, oc : oc + 1], scale=1.0,
                    )
                else:
                    nc.vector.tensor_scalar(
                        QK_bf[:, oc, nu * 512 : nu * 512 + 512],
                        ps_qk[:],
                        qb_sb[:, oc : oc + 1],
                        None,
                        op0=OP.add,
                    )

            def emit_v(mc):
                ps_v = psq.tile([128, 512], fp32, tag="q", name=f"v{mc}")
                for k in range(4):
                    nc.tensor.matmul(
                        ps_v[:],
                        xn_bf[:, k, mc * 128 : mc * 128 + 128],
                        w_bf[:, k, 2 * C : 3 * C],
                        start=(k == 0),
                        stop=(k == 3),
                    )
                nc.vector.tensor_tensor(VT_bf[:, mc, :], ps_v[:], vb_bc[:], op=OP.add)

            # ---------------- attention state ----------------
            rr = work.tile([128, 4, 512], fp32, tag="rr")
            rr_r = work.tile([128, 4, 512], fp32r, tag="rrr")
            R_sb = work.tile([128, 4, N], fp32, tag="R")
            att_un = work.tile([128, 4, N], fp32, tag="attun")
            att_bf = work.tile([128, 4, N], bf16, tag="attbf")
            part_bf = work.tile([128, 4, N], bf16, tag="part")

            def s_unit(p, s, nu):
                u = pss.tile([128, 1024], fp32, tag="s", name=f"s_p{p}s{s}n{nu}")
                for e in range(2):
                    nc.tensor.matmul(
                        u[:, e * 512 : e * 512 + 512],
                        QK_bf[e * 64 : e * 64 + 64, 4 + p, s * 128 : s * 128 + 128],
                        QK_bf[e * 64 : e * 64 + 64, p, nu * 512 : nu * 512 + 512],
                        start=True,
                        stop=True,
                        skip_group_check=True,
                    )
                return u

            def emit_proj_part(oc, nu):
                # proj partial k=0..2; bias + residual fused into the drain
                ps_o = psq.tile([128, 512], fp32, tag="q", name=f"pp{oc}n{nu}")
                for k in range(3):
                    nc.tensor.matmul(
                        ps_o[:],
                        p_bf[:, k, oc * 128 : oc * 128 + 128],
                        att_bf[:, k, nu * 512 : nu * 512 + 512],
                        start=(k == 0),
                        stop=(k == 2),
                    )
                nc.vector.scalar_tensor_tensor(
                    part_bf[:, oc, nu * 512 : nu * 512 + 512],
                    ps_o[:],
                    pb_sb[:, oc : oc + 1],
                    x_bf[:, oc, nu * 512 : nu * 512 + 512],
                    op0=OP.add,
                    op1=OP.add,
                )

            fillers = {
                (0, 0): [lambda: emit_v(3)],
                (0, 1): [lambda: emit_v(4)],
                (0, 2): [lambda: emit_v(5)],
                (0, 3): [lambda: emit_v(6)],
                (0, 4): [lambda: emit_v(7)],
                (0, 5): [lambda: emit_qk_half(1, 0)],
                (0, 6): [lambda: emit_qk_half(1, 1)],
                (1, 1): [lambda: emit_qk_half(5, 1)],
                (1, 2): [lambda: emit_qk_half(2, 0)],
                (1, 3): [lambda: emit_qk_half(2, 1)],
                (1, 4): [lambda: emit_qk_half(6, 0)],
                (1, 5): [lambda: emit_qk_half(6, 1)],
                (2, 1): [lambda: emit_qk_half(3, 0)],
                (2, 2): [lambda: emit_qk_half(3, 1)],
                (2, 3): [lambda: emit_qk_half(7, 0)],
                (2, 4): [lambda: emit_qk_half(7, 1)],
                (3, 2): [lambda: emit_proj_part(0, 0)],
                (3, 3): [lambda: emit_proj_part(0, 1)],
                (3, 4): [lambda: emit_proj_part(1, 0)],
                (3, 5): [lambda: emit_proj_part(1, 1)],
                (3, 6): [lambda: emit_proj_part(2, 0), lambda: emit_proj_part(2, 1)],
                (3, 7): [lambda: emit_proj_part(3, 0), lambda: emit_proj_part(3, 1)],
            }

            # ---------------- QKV head + first S units ----------------
            emit_qk_full(4, drain_on_act=True)
            emit_qk_full(0, drain_on_act=True)
            units = [s_unit(0, 0, 0), s_unit(0, 0, 1)]
            emit_v(0)
            emit_v(1)
            emit_v(2)
            emit_qk_half(5, 0)
            nc.gpsimd.tensor_copy(sel_r[:], sel_f[:])

            # ---------------- pipelined pair loop ----------------
            for p in range(4):
                P_bf = ppool.tile([128, 8, 2, 2, 512], bf16, tag="P")
                ps_av = psav.tile([128, 1024], fp32, tag="av")
                for s in range(8):
                    # exp: nu=0 exact on ACT, nu=1 Schraudolph on DVE
                    nc.scalar.activation(
                        P_bf[:, s, 0, :, :],
                        units[0][:].rearrange("q (h n) -> q h n", h=2),
                        AF.Exp,
                        scale=0.125,
                    )
                    nc.vector.tensor_scalar(
                        P_bf.bitcast(int16)[:, s, 1, :, :],
                        units[1][:].rearrange("q (h n) -> q h n", h=2),
                        A16, B16,
                        op0=OP.mult, op1=OP.add,
                    )
                    if s < 7:
                        units = [s_unit(p, s + 1, 0), s_unit(p, s + 1, 1)]
                    elif p < 3:
                        units = [s_unit(p + 1, 0, 0), s_unit(p + 1, 0, 1)]
                    for nu in range(2):
                        for e in range(2):
                            row = e * 64 + nu * 32
                            nc.tensor.matmul(
                                ps_r[row : row + 1, :],
                                ones_bf[:],
                                P_bf[:, s, nu, e, :],
                                start=(s == 0),
                                stop=(s == 7),
                                tile_position=(0, row),
                                skip_group_check=True,
                            )
                    for nu in range(2):
                        for e in range(2):
                            nc.tensor.matmul(
                                ps_av[e * 64 : e * 64 + 64, nu * 512 : nu * 512 + 512],
                                VT_bf[:, s, p * 128 + e * 64 : p * 128 + e * 64 + 64],
                                P_bf[:, s, nu, e, :],
                                start=(s == 0),
                                stop=(s == 7),
                                skip_group_check=True,
                            )
                    for th in fillers.get((p, s), ()):
                        th()

                # ---- pair tail ----
                nc.vector.reciprocal_approx_fast(rr[:, p, :], ps_r[:])
                nc.gpsimd.tensor_copy(rr_r[:, p, :], rr[:, p, :])
                # drain AV accumulator to SBUF (frees psav for R broadcast);
                # split halves across ACT+DVE (Pool cannot read PSUM)
                if p < 3:
                    nc.scalar.activation(att_un[:, p, 0:512], ps_av[:, 0:512],
                                         AF.Identity, bias=0.0, scale=1.0)
                    nc.vector.tensor_copy(att_un[:, p, 512:1024],
                                          ps_av[:, 512:1024])
                else:
                    nc.scalar.activation(att_un[:, p, 0:512], ps_av[:, 0:512],
                                         AF.Identity, bias=0.0, scale=1.0)
                    nc.vector.tensor_copy(att_un[:, p, 512:1024],
                                          ps_av[:, 512:1024])
                # broadcast rr rows across partitions: R = SEL^T @ rr (fp32r)
                ps_R = psav.tile([128, 1024], fp32, tag="av", name=f"R{p}")
                for nu in range(2):
                    nc.tensor.matmul(
                        ps_R[:, nu * 512 : nu * 512 + 512],
                        sel_r[:, nu, :],
                        rr_r[:, p, :],
                        start=True,
                        stop=True,
                    )
                if p == 3:
                    # keep PE busy through the tail chain so HAM stays warm
                    junk3 = psq.tile([128, 512], fp32, tag="q", name="warmup3")
                    for _ in range(6):
                        nc.tensor.matmul(
                            junk3[:], x_bf[:, 0, 0:128], x_bf[:, 0, 0:512],
                            start=True, stop=True,
                        )
                    nc.scalar.activation(R_sb[:, p, 0:512], ps_R[:, 0:512],
                                         AF.Identity, bias=0.0, scale=1.0)
                    nc.vector.tensor_copy(R_sb[:, p, 512:1024],
                                          ps_R[:, 512:1024])
                else:
                    nc.scalar.activation(R_sb[:, p, 0:512], ps_R[:, 0:512],
                                         AF.Identity, bias=0.0, scale=1.0)
                    nc.vector.tensor_copy(R_sb[:, p, 512:1024],
                                          ps_R[:, 512:1024])
                # normalize off-path: pairs 0-2 on Pool; pair 3 on DVE
                # (pair 3 gates the k3 tail)
                if p < 3:
                    nc.gpsimd.tensor_tensor(
                        att_bf[:, p, :], att_un[:, p, :], R_sb[:, p, :], op=OP.mult
                    )
                else:
                    nc.vector.tensor_tensor(
                        att_bf[:, p, :], att_un[:, p, :], R_sb[:, p, :], op=OP.mult
                    )

            # ---------------- proj k=3 + identity(part) + output ------------
            y_engs = [nc.sync, nc.scalar, nc.gpsimd, nc.sync]

            def k3_mms(ps, oc, nu):
                nc.tensor.matmul(
                    ps,
                    p_bf[:, 3, oc * 128 : oc * 128 + 128],
                    att_bf[:, 3, nu * 512 : nu * 512 + 512],
                    start=True,
                    stop=False,
                )
                nc.tensor.matmul(
                    ps,
                    id_sb[:],
                    part_bf[:, oc, nu * 512 : nu * 512 + 512],
                    start=False,
                    stop=True,
                )

            def emit_y_half(oc, nu, y_sb):
                y_engs[(oc + nu) % 4].dma_start(
                    y_d.ap().rearrange("(j p) n -> j p n", p=128)[
                        oc, :, nu * 512 : nu * 512 + 512
                    ],
                    y_sb[:, nu * 512 : nu * 512 + 512],
                )

            # oc0/oc1 via pss, oc3 via psav, oc2 via psq halves; drains split
            # per 512-half across ACT/DVE/Pool for parallelism
            for oc in (0, 1):
                ps_o = pss.tile([128, 1024], fp32, tag="s", name=f"k3_{oc}")
                for nu in range(2):
                    k3_mms(ps_o[:, nu * 512 : nu * 512 + 512], oc, nu)
                y_sb = ypool.tile([128, 1024], fp32, tag="y")
                if oc == 0:
                    nc.scalar.activation(y_sb[:, 0:512], ps_o[:, 0:512],
                                         AF.Identity, bias=0.0, scale=1.0)
                    nc.vector.tensor_copy(y_sb[:, 512:1024], ps_o[:, 512:1024])
                else:
                    nc.vector.tensor_copy(y_sb[:, 0:512], ps_o[:, 0:512])
                    nc.scalar.activation(y_sb[:, 512:1024], ps_o[:, 512:1024],
                                         AF.Identity, bias=0.0, scale=1.0)
                for nu in range(2):
                    emit_y_half(oc, nu, y_sb)
            ps_o3 = psav.tile([128, 1024], fp32, tag="av", name="k3_3")
            for nu in range(2):
                k3_mms(ps_o3[:, nu * 512 : nu * 512 + 512], 3, nu)
            y3 = ypool.tile([128, 1024], fp32, tag="y")
            nc.vector.tensor_copy(y3[:, 0:512], ps_o3[:, 0:512])
            nc.scalar.activation(y3[:, 512:1024], ps_o3[:, 512:1024],
                                 AF.Identity, bias=0.0, scale=1.0)
            for nu in range(2):
                emit_y_half(3, nu, y3)
            y2 = ypool.tile([128, 1024], fp32, tag="y")
            for nu in range(2):
                ps_o2 = psq.tile([128, 512], fp32, tag="q", name=f"k3_2n{nu}")
                k3_mms(ps_o2[:], 2, nu)
                nc.scalar.activation(y2[:, nu * 512 : nu * 512 + 512], ps_o2[:],
                                     AF.Identity, bias=0.0, scale=1.0)
                emit_y_half(2, nu, y2)

    nc.compile()
    return nc


def _get_nc(debug=False):
    if "nc" not in _cache:
        _cache["nc"] = _build_bass()
    return _cache["nc"]


def _host_inputs(x, norm_w, norm_b, qkv_w, qkv_b, proj_w, proj_b):
    bf = ml_dtypes.bfloat16
    x = np.asarray(x, dtype=np.float32).reshape(B, C, N).astype(bf)
    w16 = np.ascontiguousarray(np.asarray(qkv_w, dtype=np.float32).T).astype(bf)
    p16 = np.ascontiguousarray(np.asarray(proj_w, dtype=np.float32).T).astype(bf)
    ident16 = np.eye(128, dtype=np.float32).astype(bf)
    # selmat[row, nu, ch] = 1 iff row == (ch//64)*64 + nu*32  (R broadcast)
    selmat = np.zeros((128, 2, 128), dtype=np.float32)
    for nu in range(2):
        for ch in range(128):
            selmat[(ch // 64) * 64 + nu * 32, nu, ch] = 1.0
    G = np.zeros((128, 128), dtype=np.float32)
    GT = np.zeros((32, 4, 128), dtype=np.float32)
    for j in range(4):
        for p in range(128):
            g = 8 * j + p // 16
            G[p, j * 32 + g] = 1.0 / 16.0
            GT[g, j, p] = 1.0
    cmain = np.zeros((128, 148), dtype=np.float32)
    cmain[:, 0:128] = G
    cmain[:, 128:132] = np.asarray(norm_w, dtype=np.float32).reshape(4, 128).T
    cmain[:, 132:136] = np.asarray(norm_b, dtype=np.float32).reshape(4, 128).T
    cmain[:, 136:144] = (
        np.asarray(qkv_b, dtype=np.float32)[0 : 2 * C].reshape(8, 128).T
    )
    cmain[:, 144:148] = np.asarray(proj_b, dtype=np.float32).reshape(4, 128).T
    shared = {
        "w16": w16,
        "p16": p16,
        "ident16": ident16,
        "selmat": selmat,
        "cmain": cmain,
        "qkv_b": np.asarray(qkv_b, dtype=np.float32),
        "GTmat": GT,
    }
    in_maps = [dict(shared, x16=np.ascontiguousarray(x[i])) for i in range(B)]
    return in_maps


def kernel(x, norm_w, norm_b, qkv_w, qkv_b, proj_w, proj_b, _trace=False):
    from concourse import bass_utils

    nc = _get_nc()
    in_maps = _host_inputs(x, norm_w, norm_b, qkv_w, qkv_b, proj_w, proj_b)
    res = bass_utils.run_bass_kernel_spmd(
        nc, in_maps, core_ids=list(range(B)), trace=_trace
    )
    out = np.stack([res.results[i]["y"] for i in range(B)])
    _cache["last_result"] = res
    return out.reshape(B, C, 32, 32)
